# revision 1
# baseline (speedup 1.0000x reference)
"""DilatedAttention3D Trainium2 kernel.

Problem (hardcoded): B=1, D=H=W=32, C=512, 8 heads x 64 dims,
window sizes (8,8,8) r=1 and (16,16,16) r=2, fp32.

Sharding: each of the 8 cores owns one 16^3 block of the volume -- which is
exactly one scale-1 (16^3, r=2) window and contains eight scale-0 (8^3)
windows.  No cross-core communication is needed; the host scatters inputs and
gathers the disjoint output rows.

Math: softmax max-subtraction is skipped (scores are O(1) for this problem),
so each scale produces unnormalized u = V^T exp(S^T) and denominators
d = 1^T exp(S^T).  The reference's LSE-weighted merge of the two scales is
then exactly merged = (u0 + u1) / (d0 + d1), including the LSE_FILL=-1e8
masking of scale-1 positions a head does not cover (those get u1 = d1 = 0).

Per-core token ordering t0 = win*512 + cls*64 + m where win indexes the 8
scale-0 windows, cls the 8 dilation parity classes ((d%2,h%2,w%2)), m the 64
positions within both.  Scale-1 ordering t1 = cls*512 + win*64 + m makes each
class contiguous; head `cls` attends over exactly class `cls`.
"""

import numpy as np

import concourse.bacc as bacc
import concourse.mybir as mybir
import concourse.tile as tile
from concourse.bass_utils import run_bass_kernel_spmd

f32 = mybir.dt.float32
f32r = mybir.dt.float32r
AF = mybir.ActivationFunctionType


N_CORES = 8

# test.py hooks: set TRACE=True before calling kernel() to profile; the
# BassKernelResults lands in LAST_RESULTS.
TRACE = False
LAST_RESULTS = None

_PERMS = None
_NC_CACHE = {}

# scheduling knobs, sweepable via profile_sim
CFG = {
    "st_bufs": 2,      # (128,1024) S^T psum tiles in flight (2 banks each)
    "st_big": 0,       # 1: single (128,2048) S^T tile (4 banks, bufs=1) + one exp op
    "st_q": 0,         # >0: four (128,512) S^T tiles with bufs=st_q (1 bank each)
    "acc_bufs": 1,     # u_aug accumulator psum tiles (1 bank each)
    "mm_bufs": 3,      # proj/final psum tiles (1 bank each)
    "div_bufs": 0,     # 0 = division tail shares the mm pool
    "proj_copy": "vector",  # engine for qT/kT psum->sbuf copies
    "y_copy": "vector",     # engine for y psum->sbuf copies
    "u_copy": "vector",     # engine for u0 psum->sbuf copies
    "loop_n": 1,       # >1: wrap the whole body in a hardware loop (timing)
    "xin_bufs": 2,     # x window staging depth
    "exp_bufs": 3,     # exp output tiles
    "qk_bufs": 2,      # qT/kT/vaug tile depth
    "um_bufs": 2,      # uM merged-numerator tile depth
    "div_in_acc": 0,   # 1: dmt/rb transpose psum tiles use the acc pool
    "ds_bufs": 2,      # dstage rotation depth
    "s1_il": 0,        # 1: interleave scale-1 classes into window 0's head loop
    "qkT_bufs": 2,     # qT/kT tile rotation depth (proj-ahead horizon)
}


def _copy_op(nc, which):
    eng = CFG[which]
    if eng == "scalar":
        return nc.scalar.copy
    if eng == "any":
        return nc.any.tensor_copy
    return nc.vector.tensor_copy


def _perms():
    """perm0/perm1: (8, 4096) global flat token ids for each core's orderings."""
    global _PERMS
    if _PERMS is not None:
        return _PERMS
    d = np.arange(16)[:, None, None]
    h = np.arange(16)[None, :, None]
    w = np.arange(16)[None, None, :]
    win = (d // 8) * 4 + (h // 8) * 2 + (w // 8)
    cls = (d % 2) * 4 + (h % 2) * 2 + (w % 2)
    m = ((d % 8) // 2) * 16 + ((h % 8) // 2) * 4 + ((w % 8) // 2)
    t0 = (win * 512 + cls * 64 + m).ravel()
    t1 = (cls * 512 + win * 64 + m).ravel()
    perm0 = np.zeros((N_CORES, 4096), np.int64)
    perm1 = np.zeros((N_CORES, 4096), np.int64)
    for cid in range(N_CORES):
        wD, wH, wW = cid // 4, (cid // 2) % 2, cid % 2
        g = (((wD * 16 + d) * 32 + (wH * 16 + h)) * 32 + (wW * 16 + w)).ravel()
        perm0[cid, t0] = g
        perm1[cid, t1] = g
    _PERMS = (perm0, perm1)
    return _PERMS


def _build_nc(use_qkbias, use_obias, use_mask):
    nc = bacc.Bacc("TRN2", target_bir_lowering=False, debug=False,
                   num_devices=N_CORES)
    xt = nc.dram_tensor("xt", [512, 4096], f32r, kind="ExternalInput")
    x1t = nc.dram_tensor("x1t", [512, 4096], f32r, kind="ExternalInput")
    wq = nc.dram_tensor("wq", [512, 512], f32r, kind="ExternalInput")
    wk = nc.dram_tensor("wk", [512, 512], f32r, kind="ExternalInput")
    wv = nc.dram_tensor("wv", [512, 512], f32r, kind="ExternalInput")
    wot = nc.dram_tensor("wot", [512, 512], f32r, kind="ExternalInput")
    e8 = nc.dram_tensor("e8", [8, 512], f32r, kind="ExternalInput")
    ones1 = nc.dram_tensor("ones1", [128, 4, 8, 1], f32r, kind="ExternalInput")
    eye = nc.dram_tensor("eye", [128, 128], f32r, kind="ExternalInput")
    if use_qkbias:
        qb = nc.dram_tensor("qb", [512], f32, kind="ExternalInput")
        kb = nc.dram_tensor("kb", [512], f32, kind="ExternalInput")
    if use_obias:
        ob = nc.dram_tensor("ob", [512], f32, kind="ExternalInput")
    if use_mask:
        mb0 = nc.dram_tensor("mb0", [4096], f32, kind="ExternalInput")
        mb1 = nc.dram_tensor("mb1", [4096], f32, kind="ExternalInput")
        mk0 = nc.dram_tensor("mk0", [4096], f32, kind="ExternalInput")
    y = nc.dram_tensor("y", [4096, 512], f32, kind="ExternalOutput")

    with tile.TileContext(nc) as tc:
        with (
            tc.tile_pool(name="const", bufs=1) as cpool,
            tc.tile_pool(name="xin", bufs=CFG["xin_bufs"]) as xpool,
            tc.tile_pool(name="qkv", bufs=CFG["qk_bufs"]) as qkvpool,
            tc.tile_pool(name="expp", bufs=CFG["exp_bufs"]) as epool,
            tc.tile_pool(name="persist", bufs=1) as ppool,
            tc.tile_pool(name="merge", bufs=2) as mpool,
            tc.tile_pool(name="yout", bufs=3) as ypool,
            tc.tile_pool(name="stps", bufs=CFG["st_bufs"], space="PSUM") as stps,
            tc.tile_pool(name="accps", bufs=CFG["acc_bufs"], space="PSUM") as accps,
            tc.tile_pool(name="mmps", bufs=CFG["mm_bufs"], space="PSUM") as mmps,
            tc.tile_pool(name="divps", bufs=max(CFG["div_bufs"], 1),
                         space="PSUM") as divps,
        ):
            _divpool = (divps, "div") if CFG["div_bufs"] > 0 else (mmps, "mm")

            def _alloc_st(pool):
                if CFG["st_q"]:
                    ts = [pool.tile([128, 512], f32, name=f"stq{i}", tag="st",
                                    bufs=CFG["st_q"]) for i in range(4)]
                    return ts, lambda kc: ts[kc][:, :]
                if CFG["st_big"]:
                    t = pool.tile([128, 2048], f32, name="stb", tag="st",
                                  bufs=1)
                    return [t], lambda kc: t[:, kc * 512:(kc + 1) * 512]
                ts = [pool.tile([128, 1024], f32, name=f"st{i}", tag="st")
                      for i in range(2)]
                return ts, (lambda kc:
                            ts[kc // 2][:, (kc % 2) * 512:(kc % 2 + 1) * 512])
            from contextlib import nullcontext
            loop_ctx = (tc.For_i(0, CFG["loop_n"], 1)
                        if CFG["loop_n"] > 1 else nullcontext())
            loop_ctx.__enter__()
            # ---- constants / weights ----
            wq_sb, wk_sb, wv_sb, wot_sb = [], [], [], []
            for cc in range(4):
                for lst, src, nm in ((wq_sb, wq, "wq"), (wk_sb, wk, "wk"),
                                     (wv_sb, wv, "wv"), (wot_sb, wot, "wo")):
                    t = cpool.tile([128, 512], f32r, name=f"{nm}_sb{cc}")
                    nc.sync.dma_start(t[:], src[cc * 128:(cc + 1) * 128, :])
                    lst.append(t)
            e8_sb = cpool.tile([8, 512], f32r, name="e8_sb")
            nc.sync.dma_start(e8_sb[:], e8[:])
            eye_sb = cpool.tile([128, 128], f32r, name="eye_sb")
            nc.sync.dma_start(eye_sb[:], eye[:])
            if use_qkbias:
                qb_sb = cpool.tile([128, 4], f32, name="qb_sb")
                nc.sync.dma_start(qb_sb[:], qb.ap().rearrange("(a p) -> p a", p=128))
                kb_sb = cpool.tile([128, 4], f32, name="kb_sb")
                nc.sync.dma_start(kb_sb[:], kb.ap().rearrange("(a p) -> p a", p=128))
                # per-class layouts (partitions 0..63) for the scale-1 pass
                qb1_sb = cpool.tile([64, 8], f32, name="qb1_sb")
                nc.sync.dma_start(qb1_sb[:], qb.ap().rearrange("(a p) -> p a", p=64))
                kb1_sb = cpool.tile([64, 8], f32, name="kb1_sb")
                nc.sync.dma_start(kb1_sb[:], kb.ap().rearrange("(a p) -> p a", p=64))
            if use_obias:
                ob_sb = cpool.tile([1, 512], f32, name="ob_sb")
                nc.sync.dma_start(ob_sb[:], ob.ap().rearrange("(a) -> 1 a"))
                ones_sb = cpool.tile([1, 128], f32, name="ones_sb")
                nc.vector.memset(ones_sb[:], 1.0)
            if use_mask:
                mb0_sb = cpool.tile([128, 32], f32, name="mb0_sb")
                nc.sync.dma_start(mb0_sb[:], mb0.ap().rearrange("(a p) -> p a", p=128))
                mb1_sb = cpool.tile([128, 32], f32, name="mb1_sb")
                nc.sync.dma_start(mb1_sb[:], mb1.ap().rearrange("(a p) -> p a", p=128))
                mk0_sb = cpool.tile([128, 32], f32, name="mk0_sb")
                nc.sync.dma_start(mk0_sb[:], mk0.ap().rearrange("(a p) -> p a", p=128))

            def qbias_ap(lo, n, hc=None):
                # per-partition bias AP for hid rows [lo, lo+n) of chunk layout
                if hc is None:
                    hc, p = lo // 128, lo % 128
                else:
                    p = lo
                return qb_sb[p:p + n, hc:hc + 1]

            # ---------------- scale-1 pass (8 classes) ----------------
            # u1 for classes 2k/2k+1 shares tile k at partition offsets 0/64
            # (matching the offsets their head rows occupy in uM).
            u1_sb = [ppool.tile([128, 512], f32r, name=f"u1p{k}")
                     for k in range(4)]
            # d1 parked at partition 64 (the PE ones-row), slot c holding
            # class c's 512 denominators in (window, m) order
            d1_sb = ppool.tile([128, 8, 512], f32, name="d1_sb")

            def _scale1(c):
                x_sb = []
                for cc in range(4):
                    t = xpool.tile([128, 512], f32r, name=f"x{cc}", tag=f"x{cc}")
                    nc.sync.dma_start(t[:], x1t[cc * 128:(cc + 1) * 128,
                                                c * 512:(c + 1) * 512])
                    x_sb.append(t)
                # q1/k1: (64 hid x 512 tok)
                q1 = qkvpool.tile([64, 512], f32r, name="q1", tag="q1")
                k1 = qkvpool.tile([64, 512], f32r, name="k1", tag="k1")
                for dst, wsb, bsb in ((q1, wq_sb, "q"), (k1, wk_sb, "k")):
                    ps = mmps.tile([128, 512], f32, name="ps_qk1", tag="mm")
                    for cc in range(4):
                        nc.tensor.matmul(ps[0:64, :],
                                         wsb[cc][:, c * 64:(c + 1) * 64],
                                         x_sb[cc][:],
                                         start=(cc == 0), stop=(cc == 3))
                    if use_qkbias:
                        src = qb1_sb if bsb == "q" else kb1_sb
                        nc.vector.tensor_scalar_add(dst[:], ps[0:64, :],
                                                    src[:, c:c + 1])
                    else:
                        nc.vector.tensor_copy(dst[:], ps[0:64, :])
                # v1: project transposed (4 full-width MMs), then PE
                # transposes to natural (tok x 64) + ones col -> v1aug
                v1aug = qkvpool.tile([128, 4, 65], f32r, name="v1aug", tag="v1aug")
                v1t = qkvpool.tile([64, 512], f32r, name="v1t", tag="v1t", bufs=1)
                psv = mmps.tile([128, 512], f32, name="ps_v1t", tag="mm")
                for cc in range(4):
                    nc.tensor.matmul(psv[0:64, :],
                                     wv_sb[cc][:, c * 64:(c + 1) * 64],
                                     x_sb[cc][:],
                                     start=(cc == 0), stop=(cc == 3))
                nc.vector.tensor_copy(v1t[:], psv[0:64, :])
                for mt in range(4):
                    pst = mmps.tile([128, 64], f32r, name="ps_v1", tag="mm")
                    nc.tensor.transpose(pst[:, 0:64],
                                        v1t[:, mt * 128:(mt + 1) * 128],
                                        eye_sb[0:64, 0:64])
                    nc.vector.tensor_copy(v1aug[:, mt, 0:64], pst[:, 0:64])
                nc.sync.dma_start(v1aug[:, :, 64:65], ones1[:, :, 0, :])
                # S^T and exp
                st, stv = _alloc_st(stps)
                for kc in range(4):
                    nc.tensor.matmul(stv(kc),
                                     k1[:, kc * 128:(kc + 1) * 128],
                                     q1[:],
                                     start=True, stop=True)
                ex = epool.tile([128, 2048], f32r, name="ex", tag="ex")
                if use_mask:
                    for kc in range(4):
                        nc.scalar.activation(
                            ex[:, kc * 512:(kc + 1) * 512], stv(kc),
                            AF.Exp, bias=mb1_sb[:, c * 4 + kc:c * 4 + kc + 1])
                elif CFG["st_big"]:
                    nc.scalar.activation(ex[:], st[0][:], AF.Exp)
                elif CFG["st_q"]:
                    for kc in range(4):
                        nc.scalar.activation(ex[:, kc * 512:(kc + 1) * 512],
                                             stv(kc), AF.Exp)
                else:
                    for i in range(2):
                        nc.scalar.activation(ex[:, i * 1024:(i + 1) * 1024],
                                             st[i][:], AF.Exp)
                # u1_aug accumulation
                ua = accps.tile([128, 512], f32, name="ua", tag="uaug")
                for kc in range(4):
                    nc.tensor.matmul(ua[0:65, :],
                                     v1aug[:, kc, :],
                                     ex[:, kc * 512:(kc + 1) * 512],
                                     start=(kc == 0), stop=(kc == 3))
                # u1 lives at the same partition offset its head occupies in
                # the stacked uM tiles, so the merge adds stay
                # partition-aligned (compute engines cannot shift partitions;
                # the shift for odd classes goes through an SBUF->SBUF DMA).
                hp = (c % 2) * 64
                u1c = u1_sb[c // 2]
                if hp == 0:
                    nc.vector.tensor_copy(u1c[0:64, :], ua[0:64, :])
                else:
                    us = mpool.tile([64, 512], f32r, name="ustage", tag="ustage", bufs=1)
                    nc.vector.tensor_copy(us[:], ua[0:64, :])
                    nc.sync.dma_start(u1c[hp:hp + 64, :], us[:])
                nc.vector.tensor_copy(d1_sb[64:65, c, :], ua[64:65, :])

            if not CFG["s1_il"]:
                for c in range(8):
                    _scale1(c)

            # ---------------- scale-0 windows ----------------
            for w in range(8):
                x_sb = []
                for cc in range(4):
                    t = xpool.tile([128, 512], f32r, name=f"x{cc}", tag=f"x{cc}")
                    nc.sync.dma_start(t[:], xt[cc * 128:(cc + 1) * 128,
                                               w * 512:(w + 1) * 512])
                    x_sb.append(t)
                # qT/kT (hid x tok), 4 hid-chunks each
                qT, kT = [], []
                for hc in range(4):
                    for lst, wsb, bname in ((qT, wq_sb, "q"), (kT, wk_sb, "k")):
                        dst = qkvpool.tile([128, 512], f32r,
                                           name=f"{bname}T{hc}", tag=f"{bname}T{hc}",
                                           bufs=CFG["qkT_bufs"])
                        ps = mmps.tile([128, 512], f32, name="ps_qk", tag="mm")
                        for cc in range(4):
                            nc.tensor.matmul(ps[:],
                                             wsb[cc][:, hc * 128:(hc + 1) * 128],
                                             x_sb[cc][:],
                                             start=(cc == 0), stop=(cc == 3))
                        if use_qkbias:
                            src = qb_sb if bname == "q" else kb_sb
                            nc.vector.tensor_scalar_add(dst[:], ps[:],
                                                        src[:, hc:hc + 1])
                        else:
                            _copy_op(nc, "proj_copy")(dst[:], ps[:])
                        lst.append(dst)
                # v natural + ones -> vaug (128, kc, head, 65)
                vaug = qkvpool.tile([128, 4, 8, 65], f32r, name="vaug", tag="vaug")
                for mt in range(4):
                    ps = mmps.tile([128, 512], f32, name="ps_v", tag="mm")
                    for cc in range(4):
                        nc.tensor.matmul(ps[:],
                                         x_sb[cc][:, mt * 128:(mt + 1) * 128],
                                         wv_sb[cc][:],
                                         start=(cc == 0), stop=(cc == 3))
                    nc.vector.tensor_copy(
                        vaug[:, mt, :, 0:64],
                        ps.rearrange("p (h e) -> p h e", h=8))

                nc.sync.dma_start(vaug[:, :, :, 64:65], ones1[:])
                uM = [mpool.tile([128, 512], f32r, name=f"uM{hc}", tag=f"uM{hc}",
                                 bufs=CFG["um_bufs"])
                      for hc in range(4)]
                dM = mpool.tile([8, 512], f32, name="dM", tag="dM")
                for h in range(8):
                    hc, hp = h // 2, (h % 2) * 64
                    st, stv = _alloc_st(stps)
                    for kc in range(4):
                        nc.tensor.matmul(
                            stv(kc),
                            kT[hc][hp:hp + 64, kc * 128:(kc + 1) * 128],
                            qT[hc][hp:hp + 64, :],
                            start=True, stop=True)
                    ex = epool.tile([128, 2048], f32r, name="ex", tag="ex")
                    if use_mask:
                        for kc in range(4):
                            nc.scalar.activation(
                                ex[:, kc * 512:(kc + 1) * 512], stv(kc),
                                AF.Exp, bias=mb0_sb[:, w * 4 + kc:w * 4 + kc + 1])
                    elif CFG["st_big"]:
                        nc.scalar.activation(ex[:], st[0][:], AF.Exp)
                    elif CFG["st_q"]:
                        for kc in range(4):
                            nc.scalar.activation(
                                ex[:, kc * 512:(kc + 1) * 512], stv(kc),
                                AF.Exp)
                    else:
                        for i in range(2):
                            nc.scalar.activation(ex[:, i * 1024:(i + 1) * 1024],
                                                 st[i][:], AF.Exp)
                    ua = accps.tile([128, 512], f32, name="ua", tag="uaug")
                    for kc in range(4):
                        nc.tensor.matmul(ua[0:65, :],
                                         vaug[:, kc, h, :],
                                         ex[:, kc * 512:(kc + 1) * 512],
                                         start=(kc == 0), stop=(kc == 3))
                    if hp == 0:
                        _copy_op(nc, "u_copy")(uM[hc][0:64, :], ua[0:64, :])
                    else:
                        us = mpool.tile([64, 512], f32r, name="ustage",
                                        tag="ustage", bufs=1)
                        _copy_op(nc, "u_copy")(us[:], ua[0:64, :])
                        nc.sync.dma_start(uM[hc][hp:hp + 64, :], us[:])
                    # d row: park on partition 64, merge the scale-1 d for
                    # head h (class h covers window cols h*64..h*64+64),
                    # then DMA-shift into dM row h.
                    ds = mpool.tile([128, 512], f32, name="dstage",
                                    tag="dstage", bufs=CFG["ds_bufs"])
                    nc.vector.tensor_copy(ds[64:65, :], ua[64:65, :])
                    nc.vector.tensor_add(
                        ds[64:65, h * 64:(h + 1) * 64],
                        ds[64:65, h * 64:(h + 1) * 64],
                        d1_sb[64:65, h, w * 64:(w + 1) * 64])
                    nc.sync.dma_start(dM[h:h + 1, :], ds[64:65, :])
                    if CFG["s1_il"] and w == 0:
                        _scale1(h)

                # merge scale-1 u contributions (diagonal class blocks)
                for c in range(8):
                    hc, hp = c // 2, (c % 2) * 64
                    nc.vector.tensor_add(
                        uM[hc][hp:hp + 64, c * 64:(c + 1) * 64],
                        uM[hc][hp:hp + 64, c * 64:(c + 1) * 64],
                        u1_sb[c // 2][hp:hp + 64, w * 64:(w + 1) * 64])

                # division: recip via transposed tiles, broadcast via one-hot PE
                rT = mpool.tile([128, 32], f32, name="rT", tag="rT")
                for tcc in range(4):
                    dmt = (accps.tile([128, 8], f32, name="dmt", tag="uaug")
                           if CFG["div_in_acc"] else
                           _divpool[0].tile([128, 8], f32, name="dmt",
                                            tag=_divpool[1]))
                    nc.tensor.transpose(dmt[:, 0:8],
                                        dM[:, tcc * 128:(tcc + 1) * 128],
                                        eye_sb[0:8, 0:8].bitcast(f32))
                    nc.vector.reciprocal(rT[:, tcc * 8:(tcc + 1) * 8], dmt[:, 0:8])
                Rrow = mpool.tile([8, 512], f32r, name="Rrow", tag="Rrow")
                for tcc in range(4):
                    rb = (accps.tile([8, 128], f32, name="rb", tag="uaug")
                          if CFG["div_in_acc"] else
                          _divpool[0].tile([8, 128], f32, name="rb",
                                           tag=_divpool[1]))
                    nc.tensor.transpose(rb[0:8, :], rT[:, tcc * 8:(tcc + 1) * 8],
                                        eye_sb[:].bitcast(f32))
                    nc.vector.tensor_copy(Rrow[:, tcc * 128:(tcc + 1) * 128],
                                       rb[0:8, :])
                for hc in range(4):
                    bd = _divpool[0].tile([128, 512], f32, name="bd", tag=_divpool[1])
                    nc.tensor.matmul(bd[:], e8_sb[:, hc * 128:(hc + 1) * 128],
                                     Rrow[:], start=True, stop=True)
                    nc.vector.tensor_mul(uM[hc][:], uM[hc][:], bd[:])

                # final projection + store
                for tcc in range(4):
                    yp = mmps.tile([128, 512], f32, name="yp", tag="mm")
                    for hc in range(4):
                        nc.tensor.matmul(yp[:],
                                         uM[hc][:, tcc * 128:(tcc + 1) * 128],
                                         wot_sb[hc][:],
                                         start=(hc == 0),
                                         stop=(hc == 3 and not use_obias))
                    if use_obias:
                        nc.tensor.matmul(yp[:], ones_sb[0:1, :], ob_sb[0:1, :],
                                         start=False, stop=True)
                    y_sb = ypool.tile([128, 512], f32, name="y_sb", tag="y_sb")
                    if use_mask:
                        nc.vector.tensor_scalar_mul(
                            y_sb[:], yp[:],
                            mk0_sb[:, w * 4 + tcc:w * 4 + tcc + 1])
                    else:
                        _copy_op(nc, "y_copy")(y_sb[:], yp[:])
                    nc.sync.dma_start(
                        y[w * 512 + tcc * 128:w * 512 + (tcc + 1) * 128, :],
                        y_sb[:])
            loop_ctx.__exit__(None, None, None)

    nc.compile()
    return nc


def _get_nc(use_qkbias, use_obias, use_mask):
    key = (use_qkbias, use_obias, use_mask) + tuple(sorted(CFG.items()))
    if key not in _NC_CACHE:
        _NC_CACHE[key] = _build_nc(*key[:3])
    return _NC_CACHE[key]


def prepare(x, mask, Wq, bq, Wk, bk, Wv, bv, Wo, bo):
    """Host prep: returns (nc, in_maps) ready for run_bass_kernel_spmd."""
    x = np.ascontiguousarray(np.asarray(x, np.float32))
    mask = np.asarray(mask, np.float32)
    Wq, bq = np.asarray(Wq, np.float32), np.asarray(bq, np.float32)
    Wk, bk = np.asarray(Wk, np.float32), np.asarray(bk, np.float32)
    Wv, bv = np.asarray(Wv, np.float32), np.asarray(bv, np.float32)
    Wo, bo = np.asarray(Wo, np.float32), np.asarray(bo, np.float32)

    perm0, perm1 = _perms()
    x_flat = x.reshape(32768, 512)
    m_flat = mask.reshape(32768)

    bop = bo + Wo @ bv
    use_qkbias = bool(np.any(bq) or np.any(bk))
    use_obias = bool(np.any(bop))
    use_mask = not bool(np.all(m_flat == 1.0))

    wq_h = np.ascontiguousarray((Wq / 8.0).T, np.float32)
    wk_h = np.ascontiguousarray(Wk.T, np.float32)
    wv_h = np.ascontiguousarray(Wv.T, np.float32)
    wot_h = np.ascontiguousarray(Wo.T, np.float32)
    e8_h = np.zeros((8, 512), np.float32)
    for hc in range(4):
        p = np.arange(128)
        e8_h[2 * hc + p // 64, hc * 128 + p] = 1.0
    eye_h = np.eye(128, dtype=np.float32)
    ones_h = np.ones((128, 4, 8, 1), np.float32)

    nc = _get_nc(use_qkbias, use_obias, use_mask)

    in_maps = []
    for c in range(N_CORES):
        im = {
            "xt": np.ascontiguousarray(x_flat[perm0[c]].T),
            "x1t": np.ascontiguousarray(x_flat[perm1[c]].T),
            "wq": wq_h, "wk": wk_h, "wv": wv_h, "wot": wot_h,
            "e8": e8_h, "eye": eye_h,
            "ones1": ones_h,
        }
        if use_qkbias:
            im["qb"] = np.ascontiguousarray(bq / 8.0)
            im["kb"] = np.ascontiguousarray(bk)
        if use_obias:
            im["ob"] = np.ascontiguousarray(bop)
        if use_mask:
            im["mb0"] = np.ascontiguousarray((m_flat[perm0[c]] - 1.0) * 1e9)
            im["mb1"] = np.ascontiguousarray((m_flat[perm1[c]] - 1.0) * 1e9)
            im["mk0"] = np.ascontiguousarray(m_flat[perm0[c]])
        in_maps.append(im)
    return nc, in_maps


def kernel(**inputs):
    global LAST_RESULTS
    nc, in_maps = prepare(**inputs)
    res = run_bass_kernel_spmd(nc, in_maps, list(range(N_CORES)), trace=TRACE)
    LAST_RESULTS = res
    perm0, _ = _perms()
    out = np.zeros((32768, 512), np.float32)
    for c in range(N_CORES):
        out[perm0[c]] = res.results[c]["y"]
    return out.reshape(1, 32, 32, 32, 512)



# revision 29
# speedup vs baseline: 1.3030x; 1.3030x over previous
"""DilatedAttention3D Trainium2 kernel (v3, bf16 + fp8 q/k projections).

Problem (hardcoded): B=1, D=H=W=32, C=512, 8 heads x 64 dims,
window sizes (8,8,8) r=1 and (16,16,16) r=2, fp32 in/out.

Sharding: each of the 8 cores owns one 16^3 block of the volume -- one
scale-1 (16^3, r=2) window containing eight scale-0 (8^3) windows.  No
cross-core communication; the host scatters inputs / gathers outputs.

Math: softmax max-subtraction is skipped (scores are O(1) here), so each
scale produces unnormalized u = V^T exp(S^T) and denominators
d = 1^T exp(S^T); the reference's LSE merge is exactly
merged = (u0 + u1) / (d0 + d1).

Layout: per-core token ordering t0 = win*512 + cls*64 + m.  One
projection pass writes resident qT/kT tiles [128hid, 4096tok] (bf16);
scale-1 (class c = head c attending over dilation class c) reads them
with strided APs (cols w*512 + c*64 + m), so there is no second x load
and no separate scale-1 q/k projection.  q/k projections run in
fp8e4m3 with DoubleRow perf mode (two 128-row k-tiles per pass);
weights are pre-scaled by powers of two on the host and the
compensation is folded into the PSUM->SBUF copy.  v is projected per
scale into natural (tok x hid) layout with an appended ones-column,
giving [u; d] in one PSUM accumulation; the d row rides along into
SBUF with the u copy and is extracted SBUF->SBUF on the (otherwise
idle) GPSIMD engine, which cannot touch PSUM.  Division + output
projection of window w-1 are emitted inside window w's unit loop
(software pipelining) because each engine executes its stream in
order.  All attention matmul operands are bf16 (PSUM accumulation
fp32).
"""

import numpy as np
import ml_dtypes

import concourse.bacc as bacc
import concourse.mybir as mybir
import concourse.tile as tile
from concourse.bass_utils import run_bass_kernel_spmd

f32 = mybir.dt.float32
bf16 = mybir.dt.bfloat16
f8 = mybir.dt.float8e4
AF = mybir.ActivationFunctionType
DR = mybir.MatmulPerfMode.DoubleRow

N_CORES = 8
BF = ml_dtypes.bfloat16
F8 = ml_dtypes.float8_e4m3
QSC, KSC = 64.0, 32.0  # host pre-scale of wq/wk for fp8 range

# test.py hooks: set TRACE=True before calling kernel() to profile; the
# BassKernelResults lands in LAST_RESULTS.
TRACE = False
LAST_RESULTS = None

_PERMS = None
_NC_CACHE = {}

CFG = {
    "loop_n": 1,      # >1: wrap the body in a hardware loop (timing)
    "st_bufs": 2,     # S^T psum half-tiles in ring (2 banks each)
    "ua_bufs": 2,     # u accumulation psum tiles (1 bank each)
    "mm_bufs": 2,     # proj/outproj/bd psum tiles (1 bank each)
    "ex_bufs": 3,     # exp output tiles
    "um_bufs": 2,     # uM tile depth
    "proj_engines": ("vector", "scalar"),  # qT/kT copy rotation
    "vaug_copy": "vector",
    "u_copy": "vector",
    "y_engines": ("scalar", "scalar", "vector", "vector"),
}


def _perms():
    """perm0: (8, 4096) global flat token ids per core, t0 ordering.
    perm1 kept for the mask path (t1 = cls*512 + win*64 + m)."""
    global _PERMS
    if _PERMS is not None:
        return _PERMS
    d = np.arange(16)[:, None, None]
    h = np.arange(16)[None, :, None]
    w = np.arange(16)[None, None, :]
    win = (d // 8) * 4 + (h // 8) * 2 + (w // 8)
    cls = (d % 2) * 4 + (h % 2) * 2 + (w % 2)
    m = ((d % 8) // 2) * 16 + ((h % 8) // 2) * 4 + ((w % 8) // 2)
    t0 = (win * 512 + cls * 64 + m).ravel()
    t1 = (cls * 512 + win * 64 + m).ravel()
    perm0 = np.zeros((N_CORES, 4096), np.int64)
    perm1 = np.zeros((N_CORES, 4096), np.int64)
    for cid in range(N_CORES):
        wD, wH, wW = cid // 4, (cid // 2) % 2, cid % 2
        g = (((wD * 16 + d) * 32 + (wH * 16 + h)) * 32 + (wW * 16 + w)).ravel()
        perm0[cid, t0] = g
        perm1[cid, t1] = g
    _PERMS = (perm0, perm1)
    return _PERMS


def _eng(nc, name):
    return {"vector": nc.vector, "scalar": nc.scalar, "gpsimd": nc.gpsimd}[name]


def _copy(nc, name, dst, src):
    if name == "scalar":
        return nc.scalar.copy(dst, src)
    return _eng(nc, name).tensor_copy(dst, src)


def _build_nc(use_qkbias, use_obias, use_mask):
    nc = bacc.Bacc("TRN2", target_bir_lowering=False, debug=False,
                   num_devices=N_CORES)
    # x in t0 order: [128, cc, tok] so one DMA covers a window slice
    xt = nc.dram_tensor("xt", [128, 4, 4096], bf16, kind="ExternalInput")
    xt8 = nc.dram_tensor("xt8", [128, 4, 4096], f8, kind="ExternalInput")
    wq8 = nc.dram_tensor("wq8", [128, 4, 512], f8, kind="ExternalInput")
    wk8 = nc.dram_tensor("wk8", [128, 4, 512], f8, kind="ExternalInput")
    wv = nc.dram_tensor("wv", [512, 512], bf16, kind="ExternalInput")
    wot = nc.dram_tensor("wot", [512, 512], bf16, kind="ExternalInput")
    e8 = nc.dram_tensor("e8", [8, 512], bf16, kind="ExternalInput")
    if use_qkbias:
        qb = nc.dram_tensor("qb", [512], f32, kind="ExternalInput")
        kb = nc.dram_tensor("kb", [512], f32, kind="ExternalInput")
    if use_obias:
        ob = nc.dram_tensor("ob", [512], bf16, kind="ExternalInput")
    if use_mask:
        mb0 = nc.dram_tensor("mb0", [4096], f32, kind="ExternalInput")
        mb1 = nc.dram_tensor("mb1", [4096], f32, kind="ExternalInput")
        mk0 = nc.dram_tensor("mk0", [4096], f32, kind="ExternalInput")
    y = nc.dram_tensor("y", [4096, 512], f32, kind="ExternalOutput")

    with tile.TileContext(nc) as tc:
        with (
            tc.tile_pool(name="const", bufs=1) as cpool,
            tc.tile_pool(name="persist", bufs=1) as ppool,
            tc.tile_pool(name="expp", bufs=CFG["ex_bufs"]) as epool,
            tc.tile_pool(name="rot", bufs=2) as rpool,
            tc.tile_pool(name="stps", bufs=CFG["st_bufs"], space="PSUM") as stps,
            tc.tile_pool(name="uaps", bufs=CFG["ua_bufs"], space="PSUM") as uaps,
            tc.tile_pool(name="mmps", bufs=CFG["mm_bufs"], space="PSUM") as mmps,
        ):
            from contextlib import nullcontext
            loop_ctx = (tc.For_i(0, CFG["loop_n"], 1)
                        if CFG["loop_n"] > 1 else nullcontext())
            loop_ctx.__enter__()

            # ---- weights; wq8/wk8 first so window 0 starts early ----
            wq_sb = cpool.tile([128, 4, 512], f8, name="wq_sb")
            nc.sync.dma_start(wq_sb[:], wq8[:])
            wk_sb = cpool.tile([128, 4, 512], f8, name="wk_sb")
            nc.sync.dma_start(wk_sb[:], wk8[:])

            def _wload(src, nm):
                t = cpool.tile([128, 4, 512], bf16, name=f"{nm}_sb")
                nc.sync.dma_start(t[:], src.ap().rearrange(
                    "(c p) n -> p c n", p=128))
                return [t[:, cc, :] for cc in range(4)]

            if use_qkbias:
                qb_sb = cpool.tile([128, 4], f32, name="qb_sb")
                nc.sync.dma_start(qb_sb[:], qb.ap().rearrange("(a p) -> p a", p=128))
                kb_sb = cpool.tile([128, 4], f32, name="kb_sb")
                nc.sync.dma_start(kb_sb[:], kb.ap().rearrange("(a p) -> p a", p=128))
            if use_obias:
                ob_sb = cpool.tile([1, 512], bf16, name="ob_sb")
                nc.sync.dma_start(ob_sb[:], ob.ap().rearrange("(a) -> 1 a"))
                ones_sb = cpool.tile([1, 128], bf16, name="ones_sb")
                nc.gpsimd.memset(ones_sb[:], 1.0)
            if use_mask:
                mb0_sb = cpool.tile([128, 32], f32, name="mb0_sb")
                nc.sync.dma_start(mb0_sb[:], mb0.ap().rearrange("(a p) -> p a", p=128))
                mb1_sb = cpool.tile([128, 32], f32, name="mb1_sb")
                nc.sync.dma_start(mb1_sb[:], mb1.ap().rearrange("(a p) -> p a", p=128))
                mk0_sb = cpool.tile([128, 32], f32, name="mk0_sb")
                nc.sync.dma_start(mk0_sb[:], mk0.ap().rearrange("(a p) -> p a", p=128))

            # ---- resident big tiles ----
            xb = ppool.tile([128, 4, 4096], bf16, name="xb")
            qTb = [ppool.tile([128, 4096], bf16, name=f"qTb{hc}")
                   for hc in range(4)]
            kTb = [ppool.tile([128, 4096], bf16, name=f"kTb{hc}")
                   for hc in range(4)]
            # scale-1 v, per class: [128 t1-tok(part), 4 chunks, 64 v | ones]
            v1aug = [ppool.tile([128, 4, 65], bf16, name=f"v1aug{c}")
                     for c in range(8)]
            for c in range(8):
                nc.gpsimd.memset(v1aug[c][:, :, 64:65], 1.0)
            # scale-1 merged numerators; rows 64:128 land via stage DMA.
            # Row 64 transiently holds an even class's d row until the
            # GPSIMD extract (the WAR dep orders it before the stage DMA).
            u1_sb = [ppool.tile([128, 512], bf16, name=f"u1p{k}")
                     for k in range(4)]
            # scale-1 denominators on partition 64, slot c
            ds1 = ppool.tile([128, 8, 512], bf16, name="ds1")
            # scale-0 v per window (2 rotating persistent tiles, ones preset)
            vaugs = [ppool.tile([128, 4, 8, 65], bf16, name=f"vaug{i}")
                     for i in range(2)]
            for i in range(2):
                nc.gpsimd.memset(vaugs[i][:, :, :, 64:65], 1.0)

            # ---------------- phase P: q/k projections ----------------
            # fp8 DoubleRow: two 128-row k-tiles per pass, 2 passes per
            # 512-deep contraction; the 1/QSC (1/KSC) weight pre-scale
            # compensation is folded into the PSUM->SBUF copy.
            pe_i = 0
            wv_sb = wot_sb = e8_sb = None
            for w in range(8):
                x8 = rpool.tile([128, 4, 512], f8, name="x8", tag="x8")
                nc.sync.dma_start(x8[:], xt8[:, :, w * 512:(w + 1) * 512])
                if w == 0:
                    ws = _wload(wv, "wv") + _wload(wot, "wo")
                    wv_sb, wot_sb = ws[:4], ws[4:]
                    e8_sb = cpool.tile([8, 512], bf16, name="e8_sb")
                    nc.sync.dma_start(e8_sb[:], e8[:])
                nc.sync.dma_start(xb[:, :, w * 512:(w + 1) * 512],
                                  xt[:, :, w * 512:(w + 1) * 512])
                for hc in range(4):
                    for dstb, wsb, sc, bname in (
                            (qTb, wq_sb, 1.0 / QSC, "q"),
                            (kTb, wk_sb, 1.0 / KSC, "k")):
                        ps = mmps.tile([128, 512], f32, name="ps_qk", tag="mm")
                        for i in range(2):
                            nc.tensor.matmul(
                                ps[:],
                                wsb[:, 2 * i:2 * i + 2, hc * 128:(hc + 1) * 128],
                                x8[:, 2 * i:2 * i + 2, :],
                                start=(i == 0), stop=(i == 1),
                                perf_mode=DR)
                        dst = dstb[hc][:, w * 512:(w + 1) * 512]
                        eng = CFG["proj_engines"][pe_i % len(CFG["proj_engines"])]
                        pe_i += 1
                        if use_qkbias:
                            src = qb_sb if bname == "q" else kb_sb
                            nc.vector.tensor_scalar(
                                dst, ps[:], sc, src[:, hc:hc + 1],
                                mybir.AluOpType.mult, mybir.AluOpType.add)
                        elif eng == "scalar":
                            nc.scalar.mul(dst, ps[:], sc)
                        else:
                            _eng(nc, eng).tensor_scalar_mul(dst, ps[:], sc)

            # ---------------- phase B: scale-1 (8 classes) ----------------
            # Matmul operand APs must be contiguous (single free dim), so
            # the class-c q/k token gathers from the resident tiles go
            # through SBUF->SBUF DMAs into [64, 512] t1-ordered tiles.
            for c in range(8):
                hc, hp = c // 2, (c % 2) * 64
                kv = kTb[hc].rearrange("p (w t) -> p w t", w=8)
                qv = qTb[hc].rearrange("p (w t) -> p w t", w=8)
                qt1 = rpool.tile([64, 512], bf16, name="qt1", tag="qt1")
                nc.sync.dma_start(qt1[:],
                                  qv[hp:hp + 64, :, c * 64:(c + 1) * 64])
                kt1 = rpool.tile([64, 512], bf16, name="kt1", tag="kt1")
                nc.sync.dma_start(kt1[:],
                                  kv[hp:hp + 64, :, c * 64:(c + 1) * 64])
                # v1: natural (tok x 64), one matmul per source window so
                # the stationary x slice stays contiguous
                for q in range(4):
                    ps = mmps.tile([128, 512], f32, name="ps_v1", tag="mm")
                    for cc in range(4):
                        for j in range(2):
                            nc.tensor.matmul(
                                ps[j * 64:(j + 1) * 64, 0:64],
                                xb[:, cc, (2 * q + j) * 512 + c * 64:
                                   (2 * q + j) * 512 + (c + 1) * 64],
                                wv_sb[cc][:, c * 64:(c + 1) * 64],
                                start=(cc == 0), stop=(cc == 3))
                    nc.vector.tensor_copy(v1aug[c][:, q, 0:64], ps[:, 0:64])
                st = [stps.tile([128, 1024], f32, name=f"st{i}", tag="st")
                      for i in range(2)]
                for kc in range(4):
                    nc.tensor.matmul(
                        st[kc // 2][:, (kc % 2) * 512:(kc % 2 + 1) * 512],
                        kt1[:, kc * 128:(kc + 1) * 128],
                        qt1[:],
                        start=True, stop=True)
                ex = epool.tile([128, 2048], bf16, name="ex", tag="ex")
                if use_mask:
                    for kc in range(4):
                        nc.scalar.activation(
                            ex[:, kc * 512:(kc + 1) * 512],
                            st[kc // 2][:, (kc % 2) * 512:(kc % 2 + 1) * 512],
                            AF.Exp, bias=mb1_sb[:, c * 4 + kc:c * 4 + kc + 1])
                else:
                    for i in range(2):
                        nc.scalar.activation(ex[:, i * 1024:(i + 1) * 1024],
                                             st[i][:], AF.Exp)
                ua = uaps.tile([128, 512], f32, name="ua", tag="ua")
                for kc in range(4):
                    nc.tensor.matmul(ua[0:65, :],
                                     v1aug[c][:, kc, :],
                                     ex[:, kc * 512:(kc + 1) * 512],
                                     start=(kc == 0), stop=(kc == 3))
                # u + d-row copied together; d extracted SBUF->SBUF on
                # GPSIMD, then (odd classes) the stage DMA fills rows 64+.
                if hp == 0:
                    nc.vector.tensor_copy(u1_sb[hc][0:65, :], ua[0:65, :])
                    nc.gpsimd.tensor_copy(ds1[64:65, c, :],
                                          u1_sb[hc][64:65, :])
                else:
                    us = rpool.tile([65, 512], bf16, name="ustage",
                                    tag="ustage", bufs=2)
                    nc.vector.tensor_copy(us[:], ua[0:65, :])
                    nc.gpsimd.tensor_copy(ds1[64:65, c, :], us[64:65, :])
                    nc.sync.dma_start(u1_sb[hc][64:128, :], us[0:64, :])

            # ---------------- phase A: scale-0 windows ----------------
            # Division + out-projection of window w-1 are emitted inside
            # window w's unit loop (software pipelining): engines execute
            # their streams in order, so placing them right after window
            # w-1 would stall PE on the dM DMA -> reciprocal chain.
            def _division(st8):
                uMp, dMp = st8
                rM = rpool.tile([8, 512], bf16, name="rM", tag="rM")
                with nc.allow_low_precision(reason="bf16 reciprocal row"):
                    nc.vector.reciprocal(rM[:], dMp[:])
                for hc in range(4):
                    bd = mmps.tile([128, 512], f32, name="bd", tag="mm")
                    nc.tensor.matmul(bd[:], e8_sb[:, hc * 128:(hc + 1) * 128],
                                     rM[:], start=True, stop=True)
                    nc.vector.tensor_mul(uMp[hc][:], uMp[hc][:], bd[:])

            def _outproj(st8, wprev, tccs):
                uMp, _ = st8
                for tcc in tccs:
                    yp = mmps.tile([128, 512], f32, name="yp", tag="mm")
                    for hc in range(4):
                        nc.tensor.matmul(yp[:],
                                         uMp[hc][:, tcc * 128:(tcc + 1) * 128],
                                         wot_sb[hc][:],
                                         start=(hc == 0),
                                         stop=(hc == 3 and not use_obias))
                    if use_obias:
                        nc.tensor.matmul(yp[:], ones_sb[0:1, :], ob_sb[0:1, :],
                                         start=False, stop=True)
                    ydst = y[wprev * 512 + tcc * 128:
                             wprev * 512 + (tcc + 1) * 128, :]
                    y_sb = rpool.tile([128, 512], f32, name="y_sb",
                                      tag="y_sb", bufs=3)
                    if use_mask:
                        nc.vector.tensor_scalar_mul(
                            y_sb[:], yp[:],
                            mk0_sb[:, wprev * 4 + tcc:wprev * 4 + tcc + 1])
                    else:
                        _copy(nc, CFG["y_engines"][tcc], y_sb[:], yp[:])
                    nc.sync.dma_start(ydst, y_sb[:])

            def _vproj(w):
                vaug = vaugs[w % 2]
                for mt in range(4):
                    ps = mmps.tile([128, 512], f32, name="ps_v", tag="mm")
                    for cc in range(4):
                        nc.tensor.matmul(
                            ps[:],
                            xb[:, cc, w * 512 + mt * 128:w * 512 + (mt + 1) * 128],
                            wv_sb[cc][:],
                            start=(cc == 0), stop=(cc == 3))
                    _eng(nc, CFG["vaug_copy"]).tensor_copy(
                        vaug[:, mt, :, 0:64],
                        ps.rearrange("p (h e) -> p h e", h=8))

            prev = None
            for w in range(8):
                vaug = vaugs[w % 2]
                if w == 0:
                    _vproj(0)
                uM = [rpool.tile([128, 512], bf16, name=f"uM{hc}", tag=f"uM{hc}",
                                 bufs=CFG["um_bufs"]) for hc in range(4)]
                # d0 staging: partition 64, slot h (SBUF, filled by GPSIMD)
                dsA = rpool.tile([128, 8, 512], bf16, name="dsA", tag="dsA",
                                 bufs=2)
                for h in range(8):
                    hc, hp = h // 2, (h % 2) * 64
                    st = [stps.tile([128, 1024], f32, name=f"st{i}", tag="st")
                          for i in range(2)]
                    for kc in range(4):
                        nc.tensor.matmul(
                            st[kc // 2][:, (kc % 2) * 512:(kc % 2 + 1) * 512],
                            kTb[hc][hp:hp + 64,
                                    w * 512 + kc * 128:w * 512 + (kc + 1) * 128],
                            qTb[hc][hp:hp + 64, w * 512:(w + 1) * 512],
                            start=True, stop=True)
                    ex = epool.tile([128, 2048], bf16, name="ex", tag="ex")
                    if use_mask:
                        for kc in range(4):
                            nc.scalar.activation(
                                ex[:, kc * 512:(kc + 1) * 512],
                                st[kc // 2][:, (kc % 2) * 512:(kc % 2 + 1) * 512],
                                AF.Exp, bias=mb0_sb[:, w * 4 + kc:w * 4 + kc + 1])
                    else:
                        for i in range(2):
                            nc.scalar.activation(ex[:, i * 1024:(i + 1) * 1024],
                                                 st[i][:], AF.Exp)
                    ua = uaps.tile([128, 512], f32, name="ua", tag="ua")
                    for kc in range(4):
                        nc.tensor.matmul(ua[0:65, :],
                                         vaug[:, kc, h, :],
                                         ex[:, kc * 512:(kc + 1) * 512],
                                         start=(kc == 0), stop=(kc == 3))
                    # u + d-row in one copy; GPSIMD extracts d (SBUF->SBUF)
                    if hp == 0:
                        _copy(nc, CFG["u_copy"], uM[hc][0:65, :], ua[0:65, :])
                        nc.gpsimd.tensor_copy(dsA[64:65, h, :],
                                              uM[hc][64:65, :])
                    else:
                        us = rpool.tile([65, 512], bf16, name="ustage",
                                        tag="ustage", bufs=2)
                        _copy(nc, CFG["u_copy"], us[:], ua[0:65, :])
                        nc.gpsimd.tensor_copy(dsA[64:65, h, :], us[64:65, :])
                        nc.sync.dma_start(uM[hc][64:128, :], us[0:64, :])
                    # merge scale-1 u for class h (diagonal block)
                    nc.gpsimd.tensor_add(
                        uM[hc][hp:hp + 64, h * 64:(h + 1) * 64],
                        uM[hc][hp:hp + 64, h * 64:(h + 1) * 64],
                        u1_sb[hc][hp:hp + 64, w * 64:(w + 1) * 64])
                    # merge scale-1 d for class h (both on partition 64)
                    nc.gpsimd.tensor_add(
                        dsA[64:65, h, h * 64:(h + 1) * 64],
                        dsA[64:65, h, h * 64:(h + 1) * 64],
                        ds1[64:65, h, w * 64:(w + 1) * 64])
                    if prev is not None and h == 1:
                        _division(prev)
                    if prev is not None and h == 3:
                        _outproj(prev, w - 1, (0, 1))
                    if prev is not None and h == 5:
                        _outproj(prev, w - 1, (2, 3))
                    if h == 6 and w < 7:
                        _vproj(w + 1)

                # assemble dM [8 head, 512] via one partition-scatter DMA
                dM = rpool.tile([8, 512], bf16, name="dM", tag="dM")
                nc.sync.dma_start(dM[:], dsA[64:65, :, :])
                prev = (uM, dM)
            _division(prev)
            _outproj(prev, 7, (0, 1, 2, 3))
            loop_ctx.__exit__(None, None, None)

    nc.compile()
    return nc


def _get_nc(use_qkbias, use_obias, use_mask):
    key = (use_qkbias, use_obias, use_mask, tuple(sorted(
        (k, v if not isinstance(v, tuple) else tuple(v))
        for k, v in CFG.items())))
    if key not in _NC_CACHE:
        _NC_CACHE[key] = _build_nc(*key[:3])
    return _NC_CACHE[key]


def prepare(x, mask, Wq, bq, Wk, bk, Wv, bv, Wo, bo):
    """Host prep: returns (nc, in_maps) ready for run_bass_kernel_spmd."""
    x = np.ascontiguousarray(np.asarray(x, np.float32))
    mask = np.asarray(mask, np.float32)
    Wq, bq = np.asarray(Wq, np.float32), np.asarray(bq, np.float32)
    Wk, bk = np.asarray(Wk, np.float32), np.asarray(bk, np.float32)
    Wv, bv = np.asarray(Wv, np.float32), np.asarray(bv, np.float32)
    Wo, bo = np.asarray(Wo, np.float32), np.asarray(bo, np.float32)

    perm0, perm1 = _perms()
    x_flat = x.reshape(32768, 512)
    m_flat = mask.reshape(32768)

    bop = bo + Wo @ bv
    use_qkbias = bool(np.any(bq) or np.any(bk))
    use_obias = bool(np.any(bop))
    use_mask = not bool(np.all(m_flat == 1.0))

    # [512, n] -> [128, cc, n] chunk layout
    def chunks(a):
        return np.ascontiguousarray(
            a.reshape(4, 128, a.shape[1]).transpose(1, 0, 2))

    wq8_h = chunks((Wq.T / 8.0) * QSC).astype(F8)
    wk8_h = chunks(Wk.T * KSC).astype(F8)
    wv_h = np.ascontiguousarray(Wv.T).astype(BF)
    wot_h = np.ascontiguousarray(Wo.T).astype(BF)
    e8_h = np.zeros((8, 512), np.float32)
    for hc in range(4):
        p = np.arange(128)
        e8_h[2 * hc + p // 64, hc * 128 + p] = 1.0
    e8_h = e8_h.astype(BF)

    nc = _get_nc(use_qkbias, use_obias, use_mask)

    in_maps = []
    for c in range(N_CORES):
        xc = x_flat[perm0[c]].T  # [512, 4096]
        im = {
            "xt": chunks(xc.astype(BF)),
            "xt8": chunks(xc.astype(F8)),
            "wq8": wq8_h, "wk8": wk8_h, "wv": wv_h, "wot": wot_h,
            "e8": e8_h,
        }
        if use_qkbias:
            im["qb"] = np.ascontiguousarray(bq / 8.0)
            im["kb"] = np.ascontiguousarray(bk)
        if use_obias:
            im["ob"] = np.ascontiguousarray(bop).astype(BF)
        if use_mask:
            im["mb0"] = np.ascontiguousarray((m_flat[perm0[c]] - 1.0) * 1e9)
            im["mb1"] = np.ascontiguousarray((m_flat[perm1[c]] - 1.0) * 1e9)
            im["mk0"] = np.ascontiguousarray(m_flat[perm0[c]])
        in_maps.append(im)
    return nc, in_maps


def kernel(**inputs):
    global LAST_RESULTS
    nc, in_maps = prepare(**inputs)
    res = run_bass_kernel_spmd(nc, in_maps, list(range(N_CORES)), trace=TRACE)
    LAST_RESULTS = res
    perm0, _ = _perms()
    out = np.zeros((32768, 512), np.float32)
    for c in range(N_CORES):
        out[perm0[c]] = res.results[c]["y"]
    return out.reshape(1, 32, 32, 32, 512)
